# revision 13
# baseline (speedup 1.0000x reference)
# Trainium2 Bass kernel for the complex-valued 2-layer RNN (CRNN).
#
# Strategy: data-parallel over batch (B=128 -> 16 per core on 8 cores),
# weights replicated. All on-chip state is feature-major [feature, batch]
# so the time recurrence runs as weight-stationary matmuls with the batch
# on the moving (free) dim. Layer 1 is skewed one step behind layer 0 so
# both layers' matmuls pipeline within an iteration. PSUM accumulates the
# NEGATED pre-activation (signs baked into the stationaries and bias
# tiles) and tanh applies scale=-1; this keeps the imaginary-part minus
# signs out of the loop-carried dependency chain entirely. The input
# transform A0 = -(Mx@x + b0) is a per-chunk GEMM from streamed x, and the
# output projection y = tanh(V@h1 + bv) runs as per-chunk GEMMs; both are
# software-pipelined across loop bodies and interleaved into the step
# stream so they fill tensor-engine gaps instead of serializing at chunk
# boundaries. Biases/A0 enter PSUM through an identity-weight matmul.
import os
from contextlib import ExitStack

import numpy as np

import concourse.bass as bass
import concourse.tile as tile
from concourse import bacc, mybir
from concourse.bass import ds
from concourse.bass_utils import run_bass_kernel_spmd

T, B, IN, H, L = 2048, 128, 64, 128, 2
NCORES = 8
BL = B // NCORES            # batch per core = 16
U = 128                     # steps per chunk
BODY = 2 * U                # steps per loop body (two chunks: A, B)
NBODY = T // BODY           # loop iterations
CC = U * BL                 # columns per chunk (= 2048)
NB = CC // 512              # 512-col GEMM blocks per chunk
XCOLS = T * BL              # real x / y column count per core
XT_COLS = XCOLS + 2 * CC    # pad one body for cross-body prefetch
YPAD = CC                   # leading pad in y buffers (body-0 prev-B lands here)
YCOLS = YPAD + XCOLS + BL
F2 = 2 * H

_DT_NAME = os.environ.get("CRNN_DT", "float16")
_DT = getattr(mybir.dt, _DT_NAME)
_NPDT = {"float32": np.float32, "float16": np.float16}[_DT_NAME]

_STAT_NAMES = [
    "sI", "nU0r", "sU0i", "nU0i", "nW1r", "sW1i", "nW1i",
    "nU1r", "sU1i", "nU1i", "sMxR", "sMxI", "sVr", "sVi", "snVi",
]


def _build(dt):
    """Build the SPMD program (identical on all cores). Returns compiled nc."""
    nc = bacc.Bacc("TRN2", target_bir_lowering=False, debug=False,
                   num_devices=NCORES)
    f32 = mybir.dt.float32

    # ---- DRAM I/O ----
    xT = nc.dram_tensor("xT", [128, XT_COLS], dt, kind="ExternalInput").ap()
    carh0 = nc.dram_tensor("carh0", [128, 64], dt, kind="ExternalInput").ap()
    stat_d = {
        n: nc.dram_tensor(n, [128, 128], dt, kind="ExternalInput").ap()
        for n in _STAT_NAMES
    }
    b1t_d = nc.dram_tensor("b1t", [128, 32], dt, kind="ExternalInput").ap()
    vecs_d = nc.dram_tensor("vecs", [128, 4], f32, kind="ExternalInput").ap()
    # vecs columns: 0=-b0r 1=-b0i 2=bvr 3=bvi (per-partition bias vectors)

    yr_s = nc.dram_tensor("yr_s", [128, YCOLS], f32,
                          kind="ExternalOutput").ap()
    yi_s = nc.dram_tensor("yi_s", [128, YCOLS], f32,
                          kind="ExternalOutput").ap()
    hnf = nc.dram_tensor("hnf", [128, 64], f32, kind="ExternalOutput").ap()

    with tile.TileContext(nc) as tc, ExitStack() as ctx:
        singles = ctx.enter_context(tc.tile_pool(name="singles", bufs=1))
        ps_rec = ctx.enter_context(
            tc.tile_pool(name="ps_rec", bufs=2, space="PSUM"))
        ps_a0 = ctx.enter_context(
            tc.tile_pool(name="ps_a0", bufs=2, space="PSUM"))
        ps_y = ctx.enter_context(
            tc.tile_pool(name="ps_y", bufs=2, space="PSUM"))
        yout = ctx.enter_context(tc.tile_pool(name="yout", bufs=4))

        # ---- load constants to SBUF ----
        stat = {}
        for n in _STAT_NAMES:
            t = singles.tile([128, 128], dt, tag=n, name=n)
            nc.sync.dma_start(out=t[:], in_=stat_d[n])
            stat[n] = t
        b1t = singles.tile([128, 32], dt, tag="b1t")
        nc.sync.dma_start(out=b1t[:], in_=b1t_d)
        vecs = singles.tile([128, 4], f32, tag="vecs")
        nc.sync.dma_start(out=vecs[:], in_=vecs_d)

        x2 = singles.tile([128, 2 * CC], dt, tag="x2")           # body x
        a0c = [singles.tile([128, 2 * CC], dt, tag=f"a0c{i}",
                            name=f"a0c{i}") for i in range(2)]   # -A0 chunks
        hch = [singles.tile([128, U * 4 * BL], dt, tag=f"hch{i}",
                            name=f"hch{i}") for i in range(2)]   # h chunks
        hlast = singles.tile([128, 32], dt, tag="hlast")
        hnf32 = singles.tile([128, 64], f32, tag="hnf32")

        Tanh = mybir.ActivationFunctionType.Tanh
        TAIL = (U - 1) * 4 * BL           # offset of last step in an h chunk

        def a0_blk(ci, x2_off, blk):
            """One 512-col block of the -A0 GEMM into a0c[ci]."""
            xs = x2[:, x2_off + blk * 512: x2_off + (blk + 1) * 512]
            dst = a0c[ci].rearrange("p (s g) -> p s g", g=2 * BL)
            sb = blk * 32
            psr = ps_a0.tile([128, 512], f32, tag="psa", name="psa_r")
            nc.tensor.matmul(psr[:], stat["sMxR"][:], xs,
                             start=True, stop=True)
            nc.vector.tensor_scalar_add(
                dst[:, sb:sb + 32, 0:BL],
                psr[:].rearrange("p (s c) -> p s c", c=BL), vecs[:, 0:1])
            psi = ps_a0.tile([128, 512], f32, tag="psa", name="psa_i")
            nc.tensor.matmul(psi[:], stat["sMxI"][:], xs,
                             start=True, stop=True)
            nc.vector.tensor_scalar_add(
                dst[:, sb:sb + 32, BL:2 * BL],
                psi[:].rearrange("p (s c) -> p s c", c=BL), vecs[:, 1:2])

        def yproj_blk(ci, ybase, blk):
            """One 512-col block of y = tanh(V @ h1 + bv) from hch[ci];
            written to y-shift cols ds(ybase + blk*512, 512)."""
            hv = hch[ci][:].rearrange("p (s g) -> p s g", g=4 * BL)
            sl = slice(blk * 32, blk * 32 + 32)
            h1r = hv[:, sl, BL:2 * BL]
            h1i = hv[:, sl, 3 * BL:4 * BL]
            pr = ps_y.tile([128, 512], f32, tag="psy", name="psy_r")
            pi = ps_y.tile([128, 512], f32, tag="psy", name="psy_i")
            mm = nc.tensor.matmul
            prv = pr[:].rearrange("p (s c) -> p s c", c=BL)
            piv = pi[:].rearrange("p (s c) -> p s c", c=BL)
            mm(prv, stat["sVr"][:], h1r, start=True, stop=False)
            mm(piv, stat["sVr"][:], h1i, start=True, stop=False)
            mm(piv, stat["sVi"][:], h1r, start=False, stop=True)
            mm(prv, stat["snVi"][:], h1i, start=False, stop=True)
            tyr = yout.tile([128, 512], f32, tag="tyr", name="tyr")
            tyi = yout.tile([128, 512], f32, tag="tyi", name="tyi")
            nc.scalar.activation(tyr[:], pr[:], Tanh, bias=vecs[:, 2:3])
            nc.scalar.activation(tyi[:], pi[:], Tanh, bias=vecs[:, 3:4])
            off = blk * 512
            nc.sync.dma_start(out=yr_s[:, ds(ybase + off, 512)], in_=tyr[:])
            nc.sync.dma_start(out=yi_s[:, ds(ybase + off, 512)], in_=tyi[:])

        def step(ci, j):
            """One recurrence step (iter j of chunk ci): computes h0_t and
            (skewed) h1_{t-1}. PSUM holds the NEGATED pre-activation."""
            h = hch[ci]
            if j == 0:
                hp, po = hch[ci - 1], TAIL
            else:
                hp, po = h, (j - 1) * 4 * BL
            hpv = hp[:, po:po + 4 * BL].rearrange(
                "p (i l c) -> p i l c", i=2, l=2)
            h0ri = hpv[:, :, 0, :]
            h1ri = hpv[:, :, 1, :]
            h0r = hpv[:, 0, 0, :]
            h0i = hpv[:, 1, 0, :]
            h1r = hpv[:, 0, 1, :]
            h1i = hpv[:, 1, 1, :]

            p0 = ps_rec.tile([128, 32], f32, tag="ps0", name="ps0")
            p1 = ps_rec.tile([128, 32], f32, tag="ps1", name="ps1")
            a0v = a0c[ci][:, j * 2 * BL:(j + 1) * 2 * BL]
            mm = nc.tensor.matmul
            p0v = p0[:].rearrange("p (g c) -> p g c", c=BL)
            p1v = p1[:].rearrange("p (g c) -> p g c", c=BL)
            mm(p0[:], stat["sI"][:], a0v, start=True, stop=False)
            mm(p1[:], stat["sI"][:], b1t[:], start=True, stop=False)
            # l0: -(U0 @ h0_{t-1})
            mm(p0v, stat["nU0r"][:], h0ri, start=False, stop=False)
            mm(p0[:, 0:BL], stat["sU0i"][:], h0i, start=False, stop=False)
            mm(p0[:, BL:2 * BL], stat["nU0i"][:], h0r, start=False, stop=True)
            # l1: -(W1 @ h0_{t-1} + U1 @ h1_{t-2})
            mm(p1v, stat["nW1r"][:], h0ri, start=False, stop=False)
            mm(p1[:, 0:BL], stat["sW1i"][:], h0i, start=False, stop=False)
            mm(p1[:, BL:2 * BL], stat["nW1i"][:], h0r, start=False, stop=False)
            mm(p1v, stat["nU1r"][:], h1ri, start=False, stop=False)
            mm(p1[:, 0:BL], stat["sU1i"][:], h1i, start=False, stop=False)
            mm(p1[:, BL:2 * BL], stat["nU1i"][:], h1r, start=False, stop=True)
            hv = h[:, j * 4 * BL:(j + 1) * 4 * BL].rearrange(
                "p (g c) -> p g c", c=BL)
            hv4 = hv.rearrange("p (i l) c -> p i l c", i=2)
            nc.scalar.activation(hv4[:, :, 0, :],
                                 p0[:].rearrange("p (a c) -> p a c", c=BL),
                                 Tanh, scale=-1.0)
            nc.scalar.activation(hv4[:, :, 1, :],
                                 p1[:].rearrange("p (a c) -> p a c", c=BL),
                                 Tanh, scale=-1.0)

        # ---- prologue: initial state into hch[1] tail; A0 for body-0 A ----
        nc.sync.dma_start(out=hch[1][:, TAIL:], in_=carh0)
        nc.sync.dma_start(out=x2[:, 0:CC], in_=xT[:, 0:CC])
        for blk in range(NB):
            a0_blk(0, 0, blk)

        # ---- main loop: iv walks x columns, one body = 2*CC cols ----
        with tc.For_i(0, XCOLS, 2 * CC,
                      hint_engines=(mybir.EngineType.PE,
                                    mybir.EngineType.Activation,
                                    mybir.EngineType.DVE),
                      staggered_reset=True) as iv:
            # x for cur-B + next-A
            nc.sync.dma_start(out=x2[:], in_=xT[:, ds(iv + CC, 2 * CC)])
            # chunk A steps, interleaved with a0(cur-B) and yproj(prev-B)
            for j in range(U):
                step(0, j)
                if j % 32 == 16:
                    a0_blk(1, 0, j // 32)
                elif j % 32 == 0 and j > 0:
                    yproj_blk(1, iv, j // 32 - 1)
            yproj_blk(1, iv, NB - 1)
            # chunk B steps, interleaved with a0(next-A) and yproj(cur-A)
            for j in range(U):
                step(1, j)
                if j % 32 == 16:
                    a0_blk(0, CC, j // 32)
                elif j % 32 == 0 and j > 0:
                    yproj_blk(0, iv + CC, j // 32 - 1)
            yproj_blk(0, iv + CC, NB - 1)

        # ---- epilogue ----
        # y for the last chunk B
        for blk in range(NB):
            yproj_blk(1, XCOLS, blk)
        # h1_{T-1} (negated-psum l1 step from hch[1] tail)
        hv = hch[1][:, TAIL:].rearrange("p (i l c) -> p i l c", i=2, l=2)
        h0ri = hv[:, :, 0, :]
        h1ri = hv[:, :, 1, :]
        pe = ps_rec.tile([128, 32], f32, tag="ps0", name="ps_ep")
        pev = pe[:].rearrange("p (g c) -> p g c", c=BL)
        mm = nc.tensor.matmul
        mm(pe[:], stat["sI"][:], b1t[:], start=True, stop=False)
        mm(pev, stat["nW1r"][:], h0ri, start=False, stop=False)
        mm(pe[:, 0:BL], stat["sW1i"][:], hv[:, 1, 0, :],
           start=False, stop=False)
        mm(pe[:, BL:2 * BL], stat["nW1i"][:], hv[:, 0, 0, :],
           start=False, stop=False)
        mm(pev, stat["nU1r"][:], h1ri, start=False, stop=False)
        mm(pe[:, 0:BL], stat["sU1i"][:], hv[:, 1, 1, :],
           start=False, stop=False)
        mm(pe[:, BL:2 * BL], stat["nU1i"][:], hv[:, 0, 1, :],
           start=False, stop=True)
        nc.scalar.activation(hlast[:], pe[:], Tanh, scale=-1.0)
        # final y (step T-1)
        pr = ps_y.tile([128, BL], f32, tag="psy", name="psy_er")
        pi = ps_y.tile([128, BL], f32, tag="psy", name="psy_ei")
        mm(pr[:], stat["sVr"][:], hlast[:, 0:BL], start=True, stop=False)
        mm(pi[:], stat["sVr"][:], hlast[:, BL:2 * BL], start=True, stop=False)
        mm(pi[:], stat["sVi"][:], hlast[:, 0:BL], start=False, stop=True)
        mm(pr[:], stat["snVi"][:], hlast[:, BL:2 * BL], start=False, stop=True)
        tyr = yout.tile([128, BL], f32, tag="tyr2", name="tyr2")
        tyi = yout.tile([128, BL], f32, tag="tyi2", name="tyi2")
        nc.scalar.activation(tyr[:], pr[:], Tanh, bias=vecs[:, 2:3])
        nc.scalar.activation(tyi[:], pi[:], Tanh, bias=vecs[:, 3:4])
        nc.sync.dma_start(out=yr_s[:, YCOLS - BL:YCOLS], in_=tyr[:])
        nc.sync.dma_start(out=yi_s[:, YCOLS - BL:YCOLS], in_=tyi[:])
        # final hidden states -> fp32: [h0r | h0i | h1r | h1i] x BL
        nc.vector.tensor_copy(hnf32[:, 0:BL], hv[:, 0, 0, :])
        nc.vector.tensor_copy(hnf32[:, BL:2 * BL], hv[:, 1, 0, :])
        nc.vector.tensor_copy(hnf32[:, 2 * BL:3 * BL], hlast[:, 0:BL])
        nc.vector.tensor_copy(hnf32[:, 3 * BL:4 * BL], hlast[:, BL:2 * BL])
        nc.sync.dma_start(out=hnf, in_=hnf32[:])

    nc.compile()
    return nc


def _host_prep(inputs, npdt):
    """Per-core input arrays (weights shared, x/state sliced)."""
    f32 = np.float32
    x = np.asarray(inputs["x"], f32)
    h0all = np.asarray(inputs["h0"], f32)

    def blk(w):
        return np.asarray(w[0], f32), np.asarray(w[1], f32)

    U0r, U0i = blk(inputs["whh_w0"])
    W1r, W1i = blk(inputs["wih_w1"])
    U1r, U1i = blk(inputs["whh_w1"])
    Vr, Vi = blk(inputs["out_w"])
    Wxr, Wxi = blk(inputs["wih_w0"])
    b0 = np.concatenate([
        np.asarray(inputs["wih_b0"][0], f32) + np.asarray(inputs["whh_b0"][0], f32),
        np.asarray(inputs["wih_b0"][1], f32) + np.asarray(inputs["whh_b0"][1], f32)])
    b1 = np.concatenate([
        np.asarray(inputs["wih_b1"][0], f32) + np.asarray(inputs["whh_b1"][0], f32),
        np.asarray(inputs["wih_b1"][1], f32) + np.asarray(inputs["whh_b1"][1], f32)])
    bv = np.concatenate([np.asarray(inputs["out_b"][0], f32),
                         np.asarray(inputs["out_b"][1], f32)])

    stats = {
        "sI": np.eye(128, dtype=f32),
        "nU0r": (-U0r).T.copy(), "sU0i": U0i.T.copy(),
        "nU0i": (-U0i).T.copy(),
        "nW1r": (-W1r).T.copy(), "sW1i": W1i.T.copy(),
        "nW1i": (-W1i).T.copy(),
        "nU1r": (-U1r).T.copy(), "sU1i": U1i.T.copy(),
        "nU1i": (-U1i).T.copy(),
        # x-transform produces NEGATED A0 (negated-psum scheme)
        "sMxR": np.concatenate([-Wxr.T, Wxi.T], axis=0),
        "sMxI": np.concatenate([-Wxi.T, -Wxr.T], axis=0),
        "sVr": Vr.T.copy(), "sVi": Vi.T.copy(), "snVi": (-Vi).T.copy(),
    }
    stats = {k: v.astype(npdt) for k, v in stats.items()}

    # solve for h1_{-2} so the uniform first iteration reproduces h1_init:
    # W1b @ h0_init + U1b @ h1_m2 + b1 = atanh(h1_init)
    W1b = np.block([[W1r, -W1i], [W1i, W1r]])
    U1b = np.block([[U1r, -U1i], [U1i, U1r]])
    h0i_full = h0all[0].T            # [256, B] feature-major
    h1i_full = h0all[1].T
    rhs = np.arctanh(np.clip(h1i_full, -0.999999, 0.999999)) \
        - W1b @ h0i_full - b1[:, None]
    h1m2 = np.linalg.solve(U1b, rhs)  # [256, B]

    b1t = np.empty((128, 32), f32)
    b1t[:, 0:BL] = -b1[:128, None]
    b1t[:, BL:2 * BL] = -b1[128:, None]
    vecs = np.stack([-b0[:128], -b0[128:], bv[:128], bv[128:]], axis=1)

    per_core = []
    xTf = x.transpose(2, 0, 1)      # [128feat, T, B]
    for c in range(NCORES):
        sl = slice(c * BL, (c + 1) * BL)
        m = dict(stats)
        xt = np.zeros((128, XT_COLS), npdt)
        xt[:, :XCOLS] = np.ascontiguousarray(
            xTf[:, :, sl]).reshape(128, XCOLS).astype(npdt)
        m["xT"] = xt
        carh = np.empty((128, 64), f32)
        carh[:, 0:BL] = h0i_full[:128, sl]           # h0r
        carh[:, BL:2 * BL] = h1m2[:128, sl]          # h1r (t=-2)
        carh[:, 2 * BL:3 * BL] = h0i_full[128:, sl]  # h0i
        carh[:, 3 * BL:4 * BL] = h1m2[128:, sl]      # h1i
        m["carh0"] = carh.astype(npdt)
        m["b1t"] = b1t.astype(npdt)
        m["vecs"] = vecs
        per_core.append(m)
    return per_core


def _assemble(results):
    y = np.empty((T, B, F2), np.float32)
    hn = np.empty((L, B, F2), np.float32)
    lo = YPAD + BL
    for c in range(NCORES):
        r = results[c]
        sl = slice(c * BL, (c + 1) * BL)
        yr = r["yr_s"][:, lo:lo + XCOLS].reshape(128, T, BL)
        yi = r["yi_s"][:, lo:lo + XCOLS].reshape(128, T, BL)
        y[:, sl, :128] = yr.transpose(1, 2, 0)
        y[:, sl, 128:] = yi.transpose(1, 2, 0)
        hnl = r["hnf"]
        hn[0, sl, :128] = hnl[:, 0:BL].T
        hn[0, sl, 128:] = hnl[:, BL:2 * BL].T
        hn[1, sl, :128] = hnl[:, 2 * BL:3 * BL].T
        hn[1, sl, 128:] = hnl[:, 3 * BL:4 * BL].T
    return y, hn


_NC_CACHE = {}


def kernel(**inputs):
    key = _DT_NAME
    if key not in _NC_CACHE:
        _NC_CACHE[key] = _build(_DT)
    nc = _NC_CACHE[key]
    in_maps = _host_prep(inputs, _NPDT)
    res = run_bass_kernel_spmd(nc, in_maps, list(range(NCORES)))
    return _assemble(res.results)


# revision 15
# speedup vs baseline: 1.0044x; 1.0044x over previous
# Trainium2 Bass kernel for the complex-valued 2-layer RNN (CRNN).
#
# Strategy: data-parallel over batch (B=128 -> 16 per core on 8 cores),
# weights replicated. All on-chip state is feature-major [feature, batch]
# so the time recurrence runs as weight-stationary matmuls with the batch
# on the moving (free) dim. Layer 1 is skewed one step behind layer 0 so
# both layers' matmuls pipeline within an iteration. PSUM accumulates the
# NEGATED pre-activation (signs baked into the stationaries and bias
# tiles) and tanh applies scale=-1; this keeps the imaginary-part minus
# signs out of the loop-carried dependency chain entirely. The input
# transform A0 = -(Mx@x + b0) is a per-chunk GEMM from streamed x, and the
# output projection y = tanh(V@h1 + bv) runs as per-chunk GEMMs; both are
# software-pipelined across loop bodies and interleaved into the step
# stream so they fill tensor-engine gaps instead of serializing at chunk
# boundaries. Biases/A0 enter PSUM through an identity-weight matmul.
import os
from contextlib import ExitStack

import numpy as np

import concourse.bass as bass
import concourse.tile as tile
from concourse import bacc, mybir
from concourse.bass import ds
from concourse.bass_utils import run_bass_kernel_spmd

T, B, IN, H, L = 2048, 128, 64, 128, 2
NCORES = 8
BL = B // NCORES            # batch per core = 16
U = 256                     # steps per chunk
BODY = 2 * U                # steps per loop body (two chunks: A, B)
NBODY = T // BODY           # loop iterations
CC = U * BL                 # columns per chunk (= 2048)
NB = CC // 512              # 512-col GEMM blocks per chunk
XCOLS = T * BL              # real x / y column count per core
XT_COLS = XCOLS + 2 * CC    # pad one body for cross-body prefetch
YPAD = CC                   # leading pad in y buffers (body-0 prev-B lands here)
YCOLS = YPAD + XCOLS + BL
F2 = 2 * H

_DT_NAME = os.environ.get("CRNN_DT", "float16")
_DT = getattr(mybir.dt, _DT_NAME)
_NPDT = {"float32": np.float32, "float16": np.float16}[_DT_NAME]

_STAT_NAMES = [
    "sI", "nU0r", "sU0i", "nU0i", "nW1r", "sW1i", "nW1i",
    "nU1r", "sU1i", "nU1i", "sMxR", "sMxI", "sVr", "sVi", "snVi",
]


def _build(dt):
    """Build the SPMD program (identical on all cores). Returns compiled nc."""
    nc = bacc.Bacc("TRN2", target_bir_lowering=False, debug=False,
                   num_devices=NCORES)
    f32 = mybir.dt.float32

    # ---- DRAM I/O ----
    xT = nc.dram_tensor("xT", [128, XT_COLS], dt, kind="ExternalInput").ap()
    carh0 = nc.dram_tensor("carh0", [128, 64], dt, kind="ExternalInput").ap()
    stat_d = {
        n: nc.dram_tensor(n, [128, 128], dt, kind="ExternalInput").ap()
        for n in _STAT_NAMES
    }
    b1t_d = nc.dram_tensor("b1t", [128, 32], dt, kind="ExternalInput").ap()
    vecs_d = nc.dram_tensor("vecs", [128, 4], f32, kind="ExternalInput").ap()
    # vecs columns: 0=-b0r 1=-b0i 2=bvr 3=bvi (per-partition bias vectors)

    yr_s = nc.dram_tensor("yr_s", [128, YCOLS], f32,
                          kind="ExternalOutput").ap()
    yi_s = nc.dram_tensor("yi_s", [128, YCOLS], f32,
                          kind="ExternalOutput").ap()
    hnf = nc.dram_tensor("hnf", [128, 64], f32, kind="ExternalOutput").ap()

    with tile.TileContext(nc) as tc, ExitStack() as ctx:
        singles = ctx.enter_context(tc.tile_pool(name="singles", bufs=1))
        ps_rec = ctx.enter_context(
            tc.tile_pool(name="ps_rec", bufs=2, space="PSUM"))
        ps_a0 = ctx.enter_context(
            tc.tile_pool(name="ps_a0", bufs=2, space="PSUM"))
        ps_y = ctx.enter_context(
            tc.tile_pool(name="ps_y", bufs=2, space="PSUM"))
        yout = ctx.enter_context(tc.tile_pool(name="yout", bufs=4))

        # ---- load constants to SBUF ----
        stat = {}
        for n in _STAT_NAMES:
            t = singles.tile([128, 128], dt, tag=n, name=n)
            nc.sync.dma_start(out=t[:], in_=stat_d[n])
            stat[n] = t
        b1t = singles.tile([128, 32], dt, tag="b1t")
        nc.sync.dma_start(out=b1t[:], in_=b1t_d)
        vecs = singles.tile([128, 4], f32, tag="vecs")
        nc.sync.dma_start(out=vecs[:], in_=vecs_d)

        x2 = singles.tile([128, 2 * CC], dt, tag="x2")           # body x
        a0c = [singles.tile([128, 2 * CC], dt, tag=f"a0c{i}",
                            name=f"a0c{i}") for i in range(2)]   # -A0 chunks
        hch = [singles.tile([128, U * 4 * BL], dt, tag=f"hch{i}",
                            name=f"hch{i}") for i in range(2)]   # h chunks
        hlast = singles.tile([128, 32], dt, tag="hlast")
        hnf32 = singles.tile([128, 64], f32, tag="hnf32")

        Tanh = mybir.ActivationFunctionType.Tanh
        TAIL = (U - 1) * 4 * BL           # offset of last step in an h chunk

        def a0_blk(ci, x2_off, blk):
            """One 512-col block of the -A0 GEMM into a0c[ci]."""
            xs = x2[:, x2_off + blk * 512: x2_off + (blk + 1) * 512]
            dst = a0c[ci].rearrange("p (s g) -> p s g", g=2 * BL)
            sb = blk * 32
            psr = ps_a0.tile([128, 512], f32, tag="psa", name="psa_r")
            nc.tensor.matmul(psr[:], stat["sMxR"][:], xs,
                             start=True, stop=True)
            nc.vector.tensor_scalar_add(
                dst[:, sb:sb + 32, 0:BL],
                psr[:].rearrange("p (s c) -> p s c", c=BL), vecs[:, 0:1])
            psi = ps_a0.tile([128, 512], f32, tag="psa", name="psa_i")
            nc.tensor.matmul(psi[:], stat["sMxI"][:], xs,
                             start=True, stop=True)
            nc.vector.tensor_scalar_add(
                dst[:, sb:sb + 32, BL:2 * BL],
                psi[:].rearrange("p (s c) -> p s c", c=BL), vecs[:, 1:2])

        def yproj_blk(ci, ybase, blk):
            """One 512-col block of y = tanh(V @ h1 + bv) from hch[ci];
            written to y-shift cols ds(ybase + blk*512, 512)."""
            hv = hch[ci][:].rearrange("p (s g) -> p s g", g=4 * BL)
            sl = slice(blk * 32, blk * 32 + 32)
            h1r = hv[:, sl, BL:2 * BL]
            h1i = hv[:, sl, 3 * BL:4 * BL]
            pr = ps_y.tile([128, 512], f32, tag="psy", name="psy_r")
            pi = ps_y.tile([128, 512], f32, tag="psy", name="psy_i")
            mm = nc.tensor.matmul
            prv = pr[:].rearrange("p (s c) -> p s c", c=BL)
            piv = pi[:].rearrange("p (s c) -> p s c", c=BL)
            mm(prv, stat["sVr"][:], h1r, start=True, stop=False)
            mm(piv, stat["sVr"][:], h1i, start=True, stop=False)
            mm(piv, stat["sVi"][:], h1r, start=False, stop=True)
            mm(prv, stat["snVi"][:], h1i, start=False, stop=True)
            tyr = yout.tile([128, 512], f32, tag="tyr", name="tyr")
            tyi = yout.tile([128, 512], f32, tag="tyi", name="tyi")
            nc.scalar.activation(tyr[:], pr[:], Tanh, bias=vecs[:, 2:3])
            nc.scalar.activation(tyi[:], pi[:], Tanh, bias=vecs[:, 3:4])
            off = blk * 512
            nc.sync.dma_start(out=yr_s[:, ds(ybase + off, 512)], in_=tyr[:])
            nc.sync.dma_start(out=yi_s[:, ds(ybase + off, 512)], in_=tyi[:])

        def step(ci, j):
            """One recurrence step (iter j of chunk ci): computes h0_t and
            (skewed) h1_{t-1}. PSUM holds the NEGATED pre-activation."""
            h = hch[ci]
            if j == 0:
                hp, po = hch[ci - 1], TAIL
            else:
                hp, po = h, (j - 1) * 4 * BL
            hpv = hp[:, po:po + 4 * BL].rearrange(
                "p (i l c) -> p i l c", i=2, l=2)
            h0ri = hpv[:, :, 0, :]
            h1ri = hpv[:, :, 1, :]
            h0r = hpv[:, 0, 0, :]
            h0i = hpv[:, 1, 0, :]
            h1r = hpv[:, 0, 1, :]
            h1i = hpv[:, 1, 1, :]

            p0 = ps_rec.tile([128, 32], f32, tag="ps0", name="ps0")
            p1 = ps_rec.tile([128, 32], f32, tag="ps1", name="ps1")
            a0v = a0c[ci][:, j * 2 * BL:(j + 1) * 2 * BL]
            mm = nc.tensor.matmul
            p0v = p0[:].rearrange("p (g c) -> p g c", c=BL)
            p1v = p1[:].rearrange("p (g c) -> p g c", c=BL)
            mm(p0[:], stat["sI"][:], a0v, start=True, stop=False)
            mm(p1[:], stat["sI"][:], b1t[:], start=True, stop=False)
            # l0: -(U0 @ h0_{t-1})
            mm(p0v, stat["nU0r"][:], h0ri, start=False, stop=False)
            mm(p0[:, 0:BL], stat["sU0i"][:], h0i, start=False, stop=False)
            mm(p0[:, BL:2 * BL], stat["nU0i"][:], h0r, start=False, stop=True)
            # l1: -(W1 @ h0_{t-1} + U1 @ h1_{t-2})
            mm(p1v, stat["nW1r"][:], h0ri, start=False, stop=False)
            mm(p1[:, 0:BL], stat["sW1i"][:], h0i, start=False, stop=False)
            mm(p1[:, BL:2 * BL], stat["nW1i"][:], h0r, start=False, stop=False)
            mm(p1v, stat["nU1r"][:], h1ri, start=False, stop=False)
            mm(p1[:, 0:BL], stat["sU1i"][:], h1i, start=False, stop=False)
            mm(p1[:, BL:2 * BL], stat["nU1i"][:], h1r, start=False, stop=True)
            hv = h[:, j * 4 * BL:(j + 1) * 4 * BL].rearrange(
                "p (g c) -> p g c", c=BL)
            hv4 = hv.rearrange("p (i l) c -> p i l c", i=2)
            nc.scalar.activation(hv4[:, :, 0, :],
                                 p0[:].rearrange("p (a c) -> p a c", c=BL),
                                 Tanh, scale=-1.0)
            nc.scalar.activation(hv4[:, :, 1, :],
                                 p1[:].rearrange("p (a c) -> p a c", c=BL),
                                 Tanh, scale=-1.0)

        # ---- prologue: initial state into hch[1] tail; A0 for body-0 A ----
        nc.sync.dma_start(out=hch[1][:, TAIL:], in_=carh0)
        nc.sync.dma_start(out=x2[:, 0:CC], in_=xT[:, 0:CC])
        for blk in range(NB):
            a0_blk(0, 0, blk)

        # ---- main loop: iv walks x columns, one body = 2*CC cols ----
        with tc.For_i(0, XCOLS, 2 * CC,
                      hint_engines=(mybir.EngineType.PE,
                                    mybir.EngineType.Activation,
                                    mybir.EngineType.DVE)) as iv:
            # x for cur-B + next-A
            nc.sync.dma_start(out=x2[:], in_=xT[:, ds(iv + CC, 2 * CC)])
            # chunk A steps, interleaved with a0(cur-B) and yproj(prev-B)
            for j in range(U):
                step(0, j)
                if j % 32 == 16:
                    a0_blk(1, 0, j // 32)
                elif j % 32 == 0 and j > 0:
                    yproj_blk(1, iv, j // 32 - 1)
            yproj_blk(1, iv, NB - 1)
            # chunk B steps, interleaved with a0(next-A) and yproj(cur-A)
            for j in range(U):
                step(1, j)
                if j % 32 == 16:
                    a0_blk(0, CC, j // 32)
                elif j % 32 == 0 and j > 0:
                    yproj_blk(0, iv + CC, j // 32 - 1)
            yproj_blk(0, iv + CC, NB - 1)

        # ---- epilogue ----
        # y for the last chunk B
        for blk in range(NB):
            yproj_blk(1, XCOLS, blk)
        # h1_{T-1} (negated-psum l1 step from hch[1] tail)
        hv = hch[1][:, TAIL:].rearrange("p (i l c) -> p i l c", i=2, l=2)
        h0ri = hv[:, :, 0, :]
        h1ri = hv[:, :, 1, :]
        pe = ps_rec.tile([128, 32], f32, tag="ps0", name="ps_ep")
        pev = pe[:].rearrange("p (g c) -> p g c", c=BL)
        mm = nc.tensor.matmul
        mm(pe[:], stat["sI"][:], b1t[:], start=True, stop=False)
        mm(pev, stat["nW1r"][:], h0ri, start=False, stop=False)
        mm(pe[:, 0:BL], stat["sW1i"][:], hv[:, 1, 0, :],
           start=False, stop=False)
        mm(pe[:, BL:2 * BL], stat["nW1i"][:], hv[:, 0, 0, :],
           start=False, stop=False)
        mm(pev, stat["nU1r"][:], h1ri, start=False, stop=False)
        mm(pe[:, 0:BL], stat["sU1i"][:], hv[:, 1, 1, :],
           start=False, stop=False)
        mm(pe[:, BL:2 * BL], stat["nU1i"][:], hv[:, 0, 1, :],
           start=False, stop=True)
        nc.scalar.activation(hlast[:], pe[:], Tanh, scale=-1.0)
        # final y (step T-1)
        pr = ps_y.tile([128, BL], f32, tag="psy", name="psy_er")
        pi = ps_y.tile([128, BL], f32, tag="psy", name="psy_ei")
        mm(pr[:], stat["sVr"][:], hlast[:, 0:BL], start=True, stop=False)
        mm(pi[:], stat["sVr"][:], hlast[:, BL:2 * BL], start=True, stop=False)
        mm(pi[:], stat["sVi"][:], hlast[:, 0:BL], start=False, stop=True)
        mm(pr[:], stat["snVi"][:], hlast[:, BL:2 * BL], start=False, stop=True)
        tyr = yout.tile([128, BL], f32, tag="tyr2", name="tyr2")
        tyi = yout.tile([128, BL], f32, tag="tyi2", name="tyi2")
        nc.scalar.activation(tyr[:], pr[:], Tanh, bias=vecs[:, 2:3])
        nc.scalar.activation(tyi[:], pi[:], Tanh, bias=vecs[:, 3:4])
        nc.sync.dma_start(out=yr_s[:, YCOLS - BL:YCOLS], in_=tyr[:])
        nc.sync.dma_start(out=yi_s[:, YCOLS - BL:YCOLS], in_=tyi[:])
        # final hidden states -> fp32: [h0r | h0i | h1r | h1i] x BL
        nc.vector.tensor_copy(hnf32[:, 0:BL], hv[:, 0, 0, :])
        nc.vector.tensor_copy(hnf32[:, BL:2 * BL], hv[:, 1, 0, :])
        nc.vector.tensor_copy(hnf32[:, 2 * BL:3 * BL], hlast[:, 0:BL])
        nc.vector.tensor_copy(hnf32[:, 3 * BL:4 * BL], hlast[:, BL:2 * BL])
        nc.sync.dma_start(out=hnf, in_=hnf32[:])

    nc.compile()
    return nc


def _host_prep(inputs, npdt):
    """Per-core input arrays (weights shared, x/state sliced)."""
    f32 = np.float32
    x = np.asarray(inputs["x"], f32)
    h0all = np.asarray(inputs["h0"], f32)

    def blk(w):
        return np.asarray(w[0], f32), np.asarray(w[1], f32)

    U0r, U0i = blk(inputs["whh_w0"])
    W1r, W1i = blk(inputs["wih_w1"])
    U1r, U1i = blk(inputs["whh_w1"])
    Vr, Vi = blk(inputs["out_w"])
    Wxr, Wxi = blk(inputs["wih_w0"])
    b0 = np.concatenate([
        np.asarray(inputs["wih_b0"][0], f32) + np.asarray(inputs["whh_b0"][0], f32),
        np.asarray(inputs["wih_b0"][1], f32) + np.asarray(inputs["whh_b0"][1], f32)])
    b1 = np.concatenate([
        np.asarray(inputs["wih_b1"][0], f32) + np.asarray(inputs["whh_b1"][0], f32),
        np.asarray(inputs["wih_b1"][1], f32) + np.asarray(inputs["whh_b1"][1], f32)])
    bv = np.concatenate([np.asarray(inputs["out_b"][0], f32),
                         np.asarray(inputs["out_b"][1], f32)])

    stats = {
        "sI": np.eye(128, dtype=f32),
        "nU0r": (-U0r).T.copy(), "sU0i": U0i.T.copy(),
        "nU0i": (-U0i).T.copy(),
        "nW1r": (-W1r).T.copy(), "sW1i": W1i.T.copy(),
        "nW1i": (-W1i).T.copy(),
        "nU1r": (-U1r).T.copy(), "sU1i": U1i.T.copy(),
        "nU1i": (-U1i).T.copy(),
        # x-transform produces NEGATED A0 (negated-psum scheme)
        "sMxR": np.concatenate([-Wxr.T, Wxi.T], axis=0),
        "sMxI": np.concatenate([-Wxi.T, -Wxr.T], axis=0),
        "sVr": Vr.T.copy(), "sVi": Vi.T.copy(), "snVi": (-Vi).T.copy(),
    }
    stats = {k: v.astype(npdt) for k, v in stats.items()}

    # solve for h1_{-2} so the uniform first iteration reproduces h1_init:
    # W1b @ h0_init + U1b @ h1_m2 + b1 = atanh(h1_init)
    W1b = np.block([[W1r, -W1i], [W1i, W1r]])
    U1b = np.block([[U1r, -U1i], [U1i, U1r]])
    h0i_full = h0all[0].T            # [256, B] feature-major
    h1i_full = h0all[1].T
    rhs = np.arctanh(np.clip(h1i_full, -0.999999, 0.999999)) \
        - W1b @ h0i_full - b1[:, None]
    h1m2 = np.linalg.solve(U1b, rhs)  # [256, B]

    b1t = np.empty((128, 32), f32)
    b1t[:, 0:BL] = -b1[:128, None]
    b1t[:, BL:2 * BL] = -b1[128:, None]
    vecs = np.stack([-b0[:128], -b0[128:], bv[:128], bv[128:]], axis=1)

    per_core = []
    xTf = x.transpose(2, 0, 1)      # [128feat, T, B]
    for c in range(NCORES):
        sl = slice(c * BL, (c + 1) * BL)
        m = dict(stats)
        xt = np.zeros((128, XT_COLS), npdt)
        xt[:, :XCOLS] = np.ascontiguousarray(
            xTf[:, :, sl]).reshape(128, XCOLS).astype(npdt)
        m["xT"] = xt
        carh = np.empty((128, 64), f32)
        carh[:, 0:BL] = h0i_full[:128, sl]           # h0r
        carh[:, BL:2 * BL] = h1m2[:128, sl]          # h1r (t=-2)
        carh[:, 2 * BL:3 * BL] = h0i_full[128:, sl]  # h0i
        carh[:, 3 * BL:4 * BL] = h1m2[128:, sl]      # h1i
        m["carh0"] = carh.astype(npdt)
        m["b1t"] = b1t.astype(npdt)
        m["vecs"] = vecs
        per_core.append(m)
    return per_core


def _assemble(results):
    y = np.empty((T, B, F2), np.float32)
    hn = np.empty((L, B, F2), np.float32)
    lo = YPAD + BL
    for c in range(NCORES):
        r = results[c]
        sl = slice(c * BL, (c + 1) * BL)
        yr = r["yr_s"][:, lo:lo + XCOLS].reshape(128, T, BL)
        yi = r["yi_s"][:, lo:lo + XCOLS].reshape(128, T, BL)
        y[:, sl, :128] = yr.transpose(1, 2, 0)
        y[:, sl, 128:] = yi.transpose(1, 2, 0)
        hnl = r["hnf"]
        hn[0, sl, :128] = hnl[:, 0:BL].T
        hn[0, sl, 128:] = hnl[:, BL:2 * BL].T
        hn[1, sl, :128] = hnl[:, 2 * BL:3 * BL].T
        hn[1, sl, 128:] = hnl[:, 3 * BL:4 * BL].T
    return y, hn


_NC_CACHE = {}


def kernel(**inputs):
    key = _DT_NAME
    if key not in _NC_CACHE:
        _NC_CACHE[key] = _build(_DT)
    nc = _NC_CACHE[key]
    in_maps = _host_prep(inputs, _NPDT)
    res = run_bass_kernel_spmd(nc, in_maps, list(range(NCORES)))
    return _assemble(res.results)
